# revision 9
# baseline (speedup 1.0000x reference)
"""LRU single-step kernel for 8x TRN2 NeuronCores (Bass/Tile), fp8/bf16 datapath.

Math (per batch row b, hidden h):
  out_re[b,h] = lam_re[h]*h_re[b,h] - lam_im[h]*h_im[b,h] + (x @ (scale*B_real).T)[b,h]
  out_im[b,h] = lam_im[h]*h_re[b,h] + lam_re[h]*h_im[b,h] + (x @ (scale*B_img ).T)[b,h]

Strategy: data-parallel over the batch axis (8 shards of 32768 rows). The
problem is memory-bound, so dtypes are chosen per-tensor against the 2e-2
rel-err budget. The output variance is dominated by the projection term
(gamma is log-normal: E[gamma^2] ~ 7.4, so Var[proj] ~ 3.8 vs ~0.2 for the
lambda*h terms, |lam| <= 0.87), so:
  - h_re/h_im and the lambda weights travel as fp8 (e4m3): their
    quantization error is damped by lam  -> measured rel_l2 8.7e-3;
  - x, the projection weights, and the output stay bf16 (x in fp8 would put
    ~4% error on the DOMINANT term -> 2.7e-2, over budget).
Per-core HBM traffic: 8 (x) + 16 (h) + 32 (out) = 56 MiB vs 144 MiB in f32.

On each core everything is computed in a transposed layout (hidden on
partitions, batch on the free axis). The 256 hiddens are split into 4 groups
of 64; for group g the partition layout packs re and im halves together:
partitions 0:64 <- h_re[g*64:(g+1)*64], 64:128 <- h_im[...]. With that
packing each output tile needs exactly TWO matmuls accumulated in PSUM:

  psum[j, b]    = Wp_g[i, j].T    @ x_t[i, b]      (proj_re | proj_im packed, bf16)
                + Wlam_g[p, j].T  @ hcat_g[p, b]   (block-diag lambda mix, fp8)

HBM layout is iteration-major (built host-side, where shuffling is free):
each outer iteration's input is ONE contiguous (128, 3072-byte) slab -- 2048
fp8 h values (4 groups x 512 cols) followed by the raw bytes of 512 bf16 x
values, read back via AP.bitcast -- and its output is ONE contiguous
(128, 2048) bf16 slab. Every DMA therefore moves 128 long contiguous lines
(3-4 KB per partition). Small COLS=512 iterations keep the PE's idle gaps
far below the ~3.4 us HAM re-throttle window (matmuls stay at the warm
2.4 GHz clock) and keep the pipeline fine-grained so the drain tail is
short. Loads are issued on GpSimd (SWDGE), stores on the Sync engine
(HWDGE), so store posting never blocks load descriptor generation.

PE Matmult instructions only have one sync-wait slot in codegen, so waits
are absorbed before real matmuls run (1x1 "lane absorber" matmuls per DMA'd
tile + persistent manually-rotated PSUM tiles); _split_multiwaits moves any
remaining multi-waits onto NOPs.
"""

import numpy as np

import concourse.bass as bass
import concourse.mybir as mybir
from concourse.tile import TileContext
from concourse.bass_utils import run_bass_kernel_spmd

B_SZ, IN_DIM, HID = 262144, 128, 256
N_CORES = 8
S = B_SZ // N_CORES     # 32768 rows per core
P = 128
NGRP = HID // 64        # 4 hidden groups of 64 (re+im packed per group)
COLS = 1024             # batch columns per outer iteration
OUTER = S // COLS       # 32
MMF = 512               # matmul free dim (one fp32 PSUM bank)
NBLK = COLS // MMF      # 2
BUFS = 10               # deep prefetch so loads never stall on compute

HBYTES = NGRP * COLS            # 4096 fp8 h bytes per partition per iter
SLAB = HBYTES + 2 * COLS        # + 2048 bytes of bf16 x = 6144

F32 = mybir.dt.float32
BF16 = mybir.dt.bfloat16
FP8 = mybir.dt.float8e4
NP_BF16 = mybir.dt.np(mybir.dt.bfloat16)
NP_FP8 = mybir.dt.np(mybir.dt.float8e4)

_cache = {}

# Stashed BassKernelResults from the most recent run (for test harnesses).
LAST_RESULTS = None


def _build():
    if "nc" in _cache:
        return _cache["nc"]

    nc = bass.Bass(trn_type="TRN2")

    hx = nc.dram_tensor("hx", (P, OUTER * SLAB), FP8, kind="ExternalInput")
    consts = nc.dram_tensor("consts", (P, NGRP * P), BF16, kind="ExternalInput")
    constq = nc.dram_tensor("constq", (P, NGRP * P), FP8, kind="ExternalInput")
    ocat = nc.dram_tensor("ocat", (P, OUTER * NGRP * COLS), BF16,
                          kind="ExternalOutput")

    with TileContext(nc) as tc:
        with (
            tc.tile_pool(name="cpool", bufs=1) as cpool,
            tc.tile_pool(name="hxin", bufs=BUFS) as hxin,
            tc.tile_pool(name="outp", bufs=BUFS) as outp,
            tc.tile_pool(name="psum", bufs=1, space="PSUM") as psum,
        ):
            csb = cpool.tile([P, NGRP * P], BF16)
            csq = cpool.tile([P, NGRP * P], FP8)
            nc.gpsimd.dma_start(csb[:], consts[:, :])
            nc.gpsimd.dma_start(csq[:], constq[:, :])
            # 7 persistent data PSUM tiles + 1 scratch; allocated once so no
            # TileRelease/realloc wait sets ever form on PSUM.
            ps_tiles = [psum.tile([P, MMF], F32, tag=f"ps{i}", name=f"ps{i}")
                        for i in range(7)]
            scratch = psum.tile([P, 8], F32, tag="scratch")
            _cache["ps_idx"] = 0

            def lane_absorb(tile_ap):
                # 1x1 matmul reading the freshly-DMA'd tile: carries exactly
                # one DMA-lane wait, advancing the PE's observed clock so the
                # real matmuls don't re-wait on that lane.
                nc.tensor.matmul(scratch[0:1, 0:1], tile_ap, tile_ap,
                                 start=True, stop=True, skip_group_check=True)

            def wp_g(g):
                return csb[:, g * P: (g + 1) * P]

            def wlam_g(g):
                return csq[:, g * P: (g + 1) * P]

            lane_absorb(csb[0:1, 0:1])
            lane_absorb(csq[0:1, 0:1])

            for o in range(OUTER):
                ht = hxin.tile([P, SLAB], FP8, tag="ht")
                base = o * SLAB
                nc.gpsimd.dma_start(ht[:], hx[:, base: base + SLAB])
                lane_absorb(ht[0:1, 0:1])

                ot = outp.tile([P, NGRP * COLS], BF16, tag="ot")
                xs = ht[:, HBYTES: SLAB].bitcast(BF16)   # (P, COLS) bf16

                for g in range(NGRP):
                    # Two 512-col PSUM blocks per group share each stationary
                    # weight load: Wp_g streams both x blocks, then Wlam_g
                    # streams both h blocks (interleaved accumulation groups
                    # on different PSUM banks).
                    pss = []
                    for b in range(NBLK):
                        ps = ps_tiles[_cache["ps_idx"] % 7]
                        _cache["ps_idx"] += 1
                        pss.append(ps)
                        bs = slice(b * MMF, (b + 1) * MMF)
                        nc.tensor.matmul(ps[:], wp_g(g), xs[:, bs],
                                         start=True, stop=False)
                    for b in range(NBLK):
                        hs = slice(g * COLS + b * MMF, g * COLS + (b + 1) * MMF)
                        nc.tensor.matmul(pss[b][:], wlam_g(g), ht[:, hs],
                                         start=False, stop=True)
                    for b in range(NBLK):
                        os_ = slice(g * COLS + b * MMF, g * COLS + (b + 1) * MMF)
                        # Alternate PSUM->SBUF downcast copy engines: ACT/DVE.
                        if b % 2 == 0:
                            nc.scalar.copy(ot[:, os_], pss[b][:])
                        else:
                            nc.vector.tensor_copy(ot[:, os_], pss[b][:])

                # Store on the Sync engine (HWDGE): keeps store descriptor
                # generation off the GpSimd queue so loads prefetch freely.
                obase = o * NGRP * COLS
                nc.sync.dma_start(ocat[:, obase: obase + NGRP * COLS], ot[:])

    _split_multiwaits(nc)
    _cache["nc"] = nc
    return nc


def _split_multiwaits(nc):
    """walrus codegen allows exactly one semaphore wait per instruction.
    Move all-but-one wait of every multi-wait instruction onto single-wait
    NOP instructions spliced immediately before it on the same engine
    (engines execute their stream in order, so semantics are unchanged)."""
    k = 0
    for bb in nc.m.functions[0].blocks:
        new_list = []
        for ins in bb.instructions:
            si = ins.sync_info
            if si is not None and si.on_wait and len(si.on_wait) > 1:
                for w in si.on_wait[:-1]:
                    nop = mybir.InstNoOp(
                        name=f"WN-{k}", engine=ins.engine,
                        sync_info=mybir.SyncInfo(on_wait=[w], on_update=[]),
                    )
                    k += 1
                    new_list.append(nop)
                si.on_wait = [si.on_wait[-1]]
            new_list.append(ins)
        bb.instructions[:] = new_list


def kernel(inputs, h_re, h_im, nu_log, theta_log, B_real, B_img, gamma_log):
    global LAST_RESULTS
    inputs = np.asarray(inputs, dtype=np.float32)
    h_re = np.asarray(h_re, dtype=np.float32)
    h_im = np.asarray(h_im, dtype=np.float32)
    nu_log = np.asarray(nu_log, dtype=np.float32)
    theta_log = np.asarray(theta_log, dtype=np.float32)
    B_real = np.asarray(B_real, dtype=np.float32)
    B_img = np.asarray(B_img, dtype=np.float32)
    gamma_log = np.asarray(gamma_log, dtype=np.float32)

    # Tiny parameter math on host (matches the f32 reference computation).
    mag = np.exp(-np.exp(nu_log))          # (1, H)
    theta = np.exp(theta_log)              # (1, H)
    lam_re = (mag * np.cos(theta))[0]      # (H,)
    lam_im = (mag * np.sin(theta))[0]      # (H,)
    scale = np.exp(gamma_log).T            # (H, 1)
    w_re = (scale * B_real).T              # (IN_DIM, H)
    w_im = (scale * B_img).T               # (IN_DIM, H)

    consts = np.zeros((P, NGRP * P), np.float32)   # Wp_g blocks (bf16)
    constq = np.zeros((P, NGRP * P), np.float32)   # Wlam_g blocks (fp8)
    j = np.arange(64)
    for g in range(NGRP):
        base = g * P
        hs = slice(g * 64, (g + 1) * 64)
        consts[:, base: base + 64] = w_re[:, hs]
        consts[:, base + 64: base + 128] = w_im[:, hs]
        lr = lam_re[hs]
        li = lam_im[hs]
        # Wlam_g[p, jj]: out col jj<64 is re, jj>=64 is im.
        constq[j, base + j] = lr
        constq[64 + j, base + j] = -li
        constq[j, base + 64 + j] = li
        constq[64 + j, base + 64 + j] = lr
    consts = consts.astype(NP_BF16)
    constq = constq.astype(NP_FP8)

    in_maps = []
    for core in range(N_CORES):
        sl = slice(core * S, (core + 1) * S)
        # Iteration-major slab: per iter o, per partition p, 3072 bytes:
        #   [0:2048]    fp8 h: group g at [g*512:(g+1)*512];
        #               p<64 -> h_re[o*512+c, g*64+p], p>=64 -> h_im[...]
        #   [2048:3072] raw bytes of bf16 x[o*512+c, p]
        hx = np.empty((P, OUTER * SLAB), NP_FP8)
        hx4 = hx.reshape(P, OUTER, SLAB)
        hview = hx4[:, :, :HBYTES].reshape(P, OUTER, NGRP, COLS)
        hr = h_re[sl].astype(NP_FP8).reshape(OUTER, COLS, NGRP, 64)
        hi = h_im[sl].astype(NP_FP8).reshape(OUTER, COLS, NGRP, 64)
        hview[:64] = hr.transpose(3, 0, 2, 1)
        hview[64:] = hi.transpose(3, 0, 2, 1)
        xb = np.ascontiguousarray(
            inputs[sl].astype(NP_BF16).reshape(OUTER, COLS, P).transpose(2, 0, 1))
        hx.view(np.uint8).reshape(P, OUTER, SLAB)[:, :, HBYTES:] = \
            xb.view(np.uint8).reshape(P, OUTER, 2 * COLS)
        in_maps.append({"hx": hx, "consts": consts, "constq": constq})

    nc = _build()
    res = run_bass_kernel_spmd(nc, in_maps, core_ids=list(range(N_CORES)))
    LAST_RESULTS = res

    out = np.empty((2, B_SZ, HID), np.float32)
    for core in range(N_CORES):
        sl = slice(core * S, (core + 1) * S)
        oc = res.results[core]["ocat"].reshape(P, OUTER, NGRP, COLS) \
                                      .astype(np.float32)
        # oc[p, o, g, c]: p<64 -> out_re[o*COLS+c, g*64+p], p>=64 -> out_im
        out[0, sl] = oc[:64].transpose(1, 3, 2, 0).reshape(S, HID)
        out[1, sl] = oc[64:].transpose(1, 3, 2, 0).reshape(S, HID)
    return out
